# revision 6
# baseline (speedup 1.0000x reference)
"""Trainium2 Bass kernel for nn_M10bTranslationAdapter (cross-attention adapter).

Reference computation (B=4, L=4096, S=10, H=2048):
    q = h_english @ w_q.T; k = h_lojban @ w_k.T; v = h_lojban @ w_v.T
    probs = softmax(q @ k.T / sqrt(H)); out = h_english + alpha * ((probs @ v) @ w_o.T)

Key re-association (S=10 is tiny, so fold the big projections through S):
    scores = h_english @ kq.T / sqrt(H),  kq = (h_lojban @ w_k.T) @ w_q   [B,S,H]
    delta  = probs @ vo,                  vo = (h_lojban @ w_v.T) @ w_o.T [B,S,H]
This removes both [16384,2048]x[2048,2048] matmuls (~275 GFLOP -> ~2.7 GFLOP),
making the problem purely HBM-bound.

Distribution over 8 cores:
  - h_english row-sharded (2048 rows/core; each core's rows live in one batch).
  - kq/vo prep sharded 8-way over the 2048-wide contraction dim; partials are
    combined with a ReduceScatter whose 8 slices are laid out so core i
    receives exactly the kq/vo of its batch (i//2).

v3: all layout work is host-side packing (same class as the existing weight
packing): h is shipped twice in bf16 — row-major (for the residual add) and
pre-transposed (for the scores contraction) — so the device does NO casts and
NO xbar transposes. That keeps every DMA a plain HWDGE HBM stream, avoids the
SWDGE-vs-DMA_TRANSPOSE mutual exclusion, and keeps the DMA sem lanes clean so
the ReduceScatter partials go out as early as possible. Queues:
  - Sync HWDGE: weight loads first, then the two 8 MiB h streams.
  - Scalar HWDGE: rs-partial writes, then only the per-pair Exp activations.
  - GpSimd SWDGE: collective readback + 16 cast-stores (bf16 -> f32 rows).
The PE stream is software-pipelined (delta of pair p-1 issued between scores
of pair p) so the tensor engine never waits on the Exp and stays at its
ramped 2.4 GHz p-state.
"""
import contextlib

import ml_dtypes
import numpy as np

import concourse.bass as bass_mod
import concourse.tile as tile
from concourse import bacc, mybir
from concourse.bass_utils import run_bass_kernel_spmd

H = 2048
B, L, S = 4, 4096, 10
N_CORES = 8
RPC = (B * L) // N_CORES          # rows of h_english per core = 2048
OS = H // N_CORES                 # per-core weight contraction slice = 256
SB = B * S                        # flattened (batch, s) = 40
NP = RPC // 256                   # 256-row tile-pairs per core = 8
NH = H // 128                     # 128-wide h chunks = 16
F32 = mybir.dt.float32
BF16 = mybir.dt.bfloat16

KQ_SZ = 128 * NH * S              # 20480 floats per rs slice for kq
VO_SZ = S * H                     # 20480 floats per rs slice for vo
RS_SZ = KQ_SZ + VO_SZ             # 40960

AF = mybir.ActivationFunctionType
ALU = mybir.AluOpType


def build_graph():
    nc = bacc.Bacc(None, num_devices=N_CORES)

    hl_p = nc.declare_dram_parameter("hl_p", [128, NH * SB], BF16, isOutput=False)
    w_kT_p = nc.declare_dram_parameter("w_kT_p", [128, NH * OS], BF16, isOutput=False)
    w_q_p = nc.declare_dram_parameter("w_q_p", [128, 2 * H], BF16, isOutput=False)
    w_vT_p = nc.declare_dram_parameter("w_vT_p", [128, NH * OS], BF16, isOutput=False)
    w_oT_p = nc.declare_dram_parameter("w_oT_p", [128, 2 * H], BF16, isOutput=False)
    h_rows_p = nc.declare_dram_parameter(
        "h_rows_p", [128, NP * 2 * H], BF16, isOutput=False
    )
    h_T_p = nc.declare_dram_parameter(
        "h_T_p", [128, NP * 2 * NH * 128], BF16, isOutput=False
    )
    inv_alpha = nc.declare_dram_parameter("inv_alpha10", [S, 1], F32, isOutput=False)
    out = nc.declare_dram_parameter("out", [RPC, H], F32, isOutput=True)

    with tile.TileContext(nc) as tc, contextlib.ExitStack() as ctx:
        singles = ctx.enter_context(tc.tile_pool(name="singles", bufs=1))
        wpool = ctx.enter_context(tc.tile_pool(name="wpool", bufs=1))
        ppool = ctx.enter_context(tc.tile_pool(name="ppool", bufs=1))
        dpool = ctx.enter_context(tc.tile_pool(name="dram", bufs=1, space="DRAM"))
        opool = ctx.enter_context(tc.tile_pool(name="opool", bufs=6))
        sdpool = ctx.enter_context(tc.tile_pool(name="sdpool", bufs=3))
        spool = ctx.enter_context(tc.tile_pool(name="spool", bufs=2))
        pp_s = ctx.enter_context(tc.tile_pool(name="pp_s", bufs=2, space="PSUM"))
        pp_d = ctx.enter_context(tc.tile_pool(name="pp_d", bufs=2, space="PSUM"))
        pp_d2 = ctx.enter_context(tc.tile_pool(name="pp_d2", bufs=2, space="PSUM"))

        kq_T = singles.tile([128, NH, S], BF16)     # kq_T[p, c, s] = kq[s, 128c+p]
        vo_aug = singles.tile([S, H + 1], BF16)     # vo rows + 1/alpha column

        # ---------------- warm-up collective: absorb the ncfw entry barrier ---
        wu_in = dpool.tile([N_CORES, 64], BF16, tag="wui")
        wu_out = dpool.tile([1, 64], BF16, tag="wuo")
        nc.gpsimd.collective_compute(
            "ReduceScatter",
            mybir.AluOpType.add,
            replica_groups=[list(range(N_CORES))],
            ins=[wu_in[:].opt()],
            outs=[wu_out[:].opt()],
        )

        # ---------------- weight loads first (sync HWDGE, FIFO priority) ------
        def load_w(param, shape, tag):
            t = wpool.tile(shape, BF16, tag=tag)
            nc.sync.dma_start(
                out=t[:], in_=param[:].rearrange("p (c j) -> p c j", c=shape[1])
            )
            return t

        hl = load_w(hl_p, [128, NH, SB], "hl")
        w_kT = load_w(w_kT_p, [128, NH, OS], "wk")
        w_vT = load_w(w_vT_p, [128, NH, OS], "wv")
        w_q = load_w(w_q_p, [128, 2, H], "wq")
        w_oT = load_w(w_oT_p, [128, 2, H], "wo")

        # ---------------- h streams (sync HWDGE, behind weights) --------------
        h_rows = singles.tile([128, NP, 2, H], BF16)
        h_T = singles.tile([128, NP, 2 * NH, 128], BF16)
        for half in range(2):
            nc.sync.dma_start(
                out=h_rows[:, 4 * half : 4 * (half + 1), :, :],
                in_=h_rows_p[:, 8 * H * half : 8 * H * (half + 1)].rearrange(
                    "p (n g j) -> p n g j", n=4, g=2
                ),
            )
        for half in range(2):
            nc.sync.dma_start(
                out=h_T[:, 4 * half : 4 * (half + 1), :, :],
                in_=h_T_p[:, 4096 * 4 * half : 4096 * 4 * (half + 1)].rearrange(
                    "p (n c r) -> p n c r", n=4, c=2 * NH
                ),
            )

        # ---------------- prep: kq/vo partials on PE --------------------------
        k_T = ppool.tile([128, 2, SB], BF16, tag="kT")
        v_T = ppool.tile([128, 2, SB], BF16, tag="vT")
        for w_sb, dst in ((w_kT, k_T), (w_vT, v_T)):
            for oc in range(2):
                ps = pp_s.tile([128, SB], F32, tag="s")
                for hc in range(NH):
                    nc.tensor.matmul(
                        ps[:],
                        lhsT=w_sb[:, hc, 128 * oc : 128 * (oc + 1)],
                        rhs=hl[:, hc, :],
                        start=(hc == 0),
                        stop=(hc == NH - 1),
                    )
                nc.vector.tensor_copy(dst[:, oc, :], ps[:])

        kq_pT = ppool.tile([128, B, NH, S], BF16, tag="kqp")
        for hc in range(NH):
            ps = pp_s.tile([128, SB], F32, tag="s")
            for oc in range(2):
                nc.tensor.matmul(
                    ps[:],
                    lhsT=w_q[:, oc, 128 * hc : 128 * (hc + 1)],
                    rhs=k_T[:, oc, :],
                    start=(oc == 0),
                    stop=(oc == 1),
                )
            nc.vector.tensor_copy(
                kq_pT[:, :, hc, :], ps[:].rearrange("p (b s) -> p b s", b=B)
            )

        vo_p = ppool.tile([SB, H], BF16, tag="vop")
        for n4 in range(4):
            ps = pp_s.tile([SB, 512], F32, tag="s")
            for hc2 in range(2):
                nc.tensor.matmul(
                    ps[:],
                    lhsT=v_T[:, hc2, :],
                    rhs=w_oT[:, hc2, 512 * n4 : 512 * (n4 + 1)],
                    start=(hc2 == 0),
                    stop=(hc2 == 1),
                )
            nc.vector.tensor_copy(vo_p[:, 512 * n4 : 512 * (n4 + 1)], ps[:])

        rs_in = dpool.tile([N_CORES, RS_SZ], BF16, tag="rsi")
        rs_out = dpool.tile([1, RS_SZ], BF16, tag="rso")

        # ------------- rs writes (scalar), RS, readback (gpsimd) --------------
        rs_ap = rs_in[:]
        for b in range(B):
            kq_src = kq_pT[:, b, :, :]
            kq_bcast = bass_mod.AP(
                tensor=kq_src.tensor,
                offset=kq_src.offset,
                ap=[kq_src.ap[0], [0, 2], [1, NH * S]],
            )
            kq_dst = bass_mod.AP(
                tensor=rs_ap.tensor,
                offset=rs_ap.offset + 2 * b * RS_SZ,
                ap=[[NH * S, 128], [RS_SZ, 2], [1, NH * S]],
            )
            nc.scalar.dma_start(out=kq_dst, in_=kq_bcast)
            vo_src = vo_p[S * b : S * (b + 1), :]
            vo_bcast = bass_mod.AP(
                tensor=vo_src.tensor,
                offset=vo_src.offset,
                ap=[vo_src.ap[0], [0, 2], vo_src.ap[1]],
            )
            vo_dst = bass_mod.AP(
                tensor=rs_ap.tensor,
                offset=rs_ap.offset + 2 * b * RS_SZ + KQ_SZ,
                ap=[[H, S], [RS_SZ, 2], [1, H]],
            )
            nc.scalar.dma_start(out=vo_dst, in_=vo_bcast)

        nc.gpsimd.collective_compute(
            "ReduceScatter",
            mybir.AluOpType.add,
            replica_groups=[list(range(N_CORES))],
            ins=[rs_in[:].opt()],
            outs=[rs_out[:].opt()],
        )
        nc.gpsimd.dma_start(
            out=kq_T[:],
            in_=rs_out[0, :KQ_SZ].rearrange("(p c s) -> p c s", p=128, c=NH),
        )
        nc.gpsimd.dma_start(
            out=vo_aug[:, :H],
            in_=rs_out[0, KQ_SZ:].rearrange("(s o) -> s o", s=S),
        )
        nc.gpsimd.dma_start(out=vo_aug[:, H : H + 1], in_=inv_alpha[:])

        # ------------------------- compute loop (SW-pipelined) ----------------
        # 512-row groups (2 tile-pairs each); delta of group g emitted after
        # scores of group g+1 so the PE never waits on the Exp.
        NG = NP // 2  # 4 groups

        def emit_delta(g4, exp_sT):
            for t in range(4):
                p, g = 2 * g4 + t // 2, t % 2
                out_t = opool.tile([128, H], BF16, tag="out")
                exp_t = exp_sT[:, 128 * t : 128 * (t + 1)]
                ps_d2 = pp_d2.tile([128, 1], F32, tag="d2")
                nc.tensor.matmul(
                    ps_d2[:], lhsT=exp_t, rhs=vo_aug[:, H : H + 1],
                    start=True, stop=True,
                )
                r_scale = spool.tile([128, 1], F32, tag="rs")
                nc.vector.reciprocal(r_scale[:], ps_d2[:])

                for half in range(2):
                    ps_d = pp_d.tile([128, 1024], F32, tag="d")
                    for q in range(2):
                        n4 = 2 * half + q
                        nc.tensor.matmul(
                            ps_d[:, 512 * q : 512 * (q + 1)],
                            lhsT=exp_t,
                            rhs=vo_aug[:, 512 * n4 : 512 * (n4 + 1)],
                            start=True,
                            stop=True,
                        )
                    h_res = h_rows[:, p, g, 1024 * half : 1024 * (half + 1)]
                    if t % 2 == 0:
                        # out = delta * (alpha / sum exp) + h on DVE
                        nc.vector.scalar_tensor_tensor(
                            out_t[:, 1024 * half : 1024 * (half + 1)],
                            ps_d[:],
                            r_scale[:],
                            h_res,
                            op0=ALU.mult,
                            op1=ALU.add,
                        )
                    else:
                        # scaled copy on ScalarE (PSUM port), add on GpSimd
                        sd = sdpool.tile([128, 1024], BF16, tag="sd")
                        nc.scalar.activation(
                            sd[:], ps_d[:], AF.Copy, scale=r_scale[:]
                        )
                        nc.gpsimd.tensor_tensor(
                            out_t[:, 1024 * half : 1024 * (half + 1)],
                            sd[:],
                            h_res,
                            op=ALU.add,
                        )
                # cast-store bf16 -> f32 rows (SWDGE)
                nc.gpsimd.dma_start(
                    out=out[512 * g4 + 128 * t : 512 * g4 + 128 * (t + 1), :],
                    in_=out_t[:],
                )

        prev = None
        for g4 in range(NG):
            hT_g = h_T[:, 2 * g4 : 2 * g4 + 2, :, :].rearrange(
                "p n (g c) r -> p c n g r", g=2
            )
            ps_s = pp_s.tile([S, 512], F32, tag="s")
            for hc in range(NH):
                nc.tensor.matmul(
                    ps_s[:],
                    lhsT=kq_T[:, hc, :],
                    rhs=hT_g[:, hc, :, :, :],
                    start=(hc == 0),
                    stop=(hc == NH - 1),
                )
            exp_sT = spool.tile([S, 512], BF16, tag="exp")
            nc.scalar.activation(
                exp_sT[:], ps_s[:], AF.Exp, scale=float(1.0 / np.sqrt(H))
            )
            if prev is not None:
                emit_delta(*prev)
            prev = (g4, exp_sT)
        emit_delta(*prev)

    nc.compile()
    return nc


_graph_cache = {}


def _get_graph():
    if "nc" not in _graph_cache:
        _graph_cache["nc"] = build_graph()
    return _graph_cache["nc"]


def _pack(x):
    """[C*128, J] f32 -> [128, C*J] partition-major packed bf16."""
    c = x.shape[0] // 128
    return np.ascontiguousarray(
        x.reshape(c, 128, x.shape[1]).transpose(1, 0, 2).reshape(128, -1)
    ).astype(ml_dtypes.bfloat16)


def _make_in_maps(inputs):
    h_english = np.ascontiguousarray(np.asarray(inputs["h_english"], dtype=np.float32))
    h_lojban = np.ascontiguousarray(np.asarray(inputs["h_lojban"], dtype=np.float32))
    w_q = np.asarray(inputs["w_q"], dtype=np.float32)
    w_k = np.asarray(inputs["w_k"], dtype=np.float32)
    w_v = np.asarray(inputs["w_v"], dtype=np.float32)
    w_o = np.asarray(inputs["w_o"], dtype=np.float32)
    alpha = float(np.asarray(inputs["alpha"], dtype=np.float32))

    h_flat = h_english.reshape(B * L, H)
    hl_p = _pack(np.ascontiguousarray(h_lojban.reshape(SB, H).T))
    inv_a = np.full((S, 1), 1.0 / alpha, dtype=np.float32)

    in_maps = []
    for i in range(N_CORES):
        sl = slice(OS * i, OS * (i + 1))
        hc = h_flat[RPC * i : RPC * (i + 1)]
        # rows: [p2, pair, g, :] = h row (256*pair + 128*g + p2)
        h_rows = np.ascontiguousarray(
            hc.reshape(NP, 2, 128, H).transpose(2, 0, 1, 3).reshape(128, -1)
        ).astype(ml_dtypes.bfloat16)
        # transposed: [p2, pair, g*16+hck, r] = h[256*pair+128*g+r, 128*hck+p2]
        h_T = np.ascontiguousarray(
            hc.reshape(NP, 2, 128, NH, 128).transpose(4, 0, 1, 3, 2).reshape(128, -1)
        ).astype(ml_dtypes.bfloat16)
        in_maps.append({
            "hl_p": hl_p,
            "w_kT_p": _pack(np.ascontiguousarray(w_k[sl, :].T)),
            "w_q_p": _pack(np.ascontiguousarray(w_q[sl, :])),
            "w_vT_p": _pack(np.ascontiguousarray(w_v[sl, :].T)),
            "w_oT_p": _pack(np.ascontiguousarray(w_o[:, sl].T)),
            "h_rows_p": h_rows,
            "h_T_p": h_T,
            "inv_alpha10": inv_a,
        })
    return in_maps


def kernel(**inputs):
    in_maps = _make_in_maps(inputs)
    nc = _get_graph()
    res = run_bass_kernel_spmd(nc, in_maps, core_ids=list(range(N_CORES)))
    out = np.concatenate([res.results[i]["out"] for i in range(N_CORES)], axis=0)
    return np.ascontiguousarray(out.reshape(B, L, H).astype(np.float32))


# revision 8
# speedup vs baseline: 1.3274x; 1.3274x over previous
"""Trainium2 Bass kernel for nn_M10bTranslationAdapter (cross-attention adapter).

Reference computation (B=4, L=4096, S=10, H=2048):
    q = h_english @ w_q.T; k = h_lojban @ w_k.T; v = h_lojban @ w_v.T
    probs = softmax(q @ k.T / sqrt(H)); out = h_english + alpha * ((probs @ v) @ w_o.T)

Key re-association (S=10 is tiny, so fold the big projections through S):
    scores = h_english @ kq.T / sqrt(H),  kq = (h_lojban @ w_k.T) @ w_q   [B,S,H]
    delta  = probs @ vo,                  vo = (h_lojban @ w_v.T) @ w_o.T [B,S,H]
This removes both [16384,2048]x[2048,2048] matmuls (~275 GFLOP -> ~2.7 GFLOP),
making the problem purely HBM-bound.

Distribution over 8 cores:
  - h_english row-sharded (2048 rows/core; each core's rows live in one batch).
  - kq/vo prep sharded 8-way over the 2048-wide contraction dim; partials are
    combined with a ReduceScatter whose 8 slices are laid out so core i
    receives exactly the kq/vo of its batch (i//2).

v3: all layout work is host-side packing (same class as the existing weight
packing): h is shipped twice in bf16 — row-major (for the residual add) and
pre-transposed (for the scores contraction) — so the device does NO casts and
NO xbar transposes. That keeps every DMA a plain HWDGE HBM stream, avoids the
SWDGE-vs-DMA_TRANSPOSE mutual exclusion, and keeps the DMA sem lanes clean so
the ReduceScatter partials go out as early as possible. Queues:
  - Sync HWDGE: weight loads first, then the two 8 MiB h streams.
  - Scalar HWDGE: rs-partial writes, then only the per-pair Exp activations.
  - GpSimd SWDGE: collective readback + 16 cast-stores (bf16 -> f32 rows).
The PE stream is software-pipelined (delta of pair p-1 issued between scores
of pair p) so the tensor engine never waits on the Exp and stays at its
ramped 2.4 GHz p-state.
"""
import contextlib

import ml_dtypes
import numpy as np

import concourse.bass as bass_mod
import concourse.tile as tile
from concourse import bacc, mybir
from concourse.bass_utils import run_bass_kernel_spmd

H = 2048
B, L, S = 4, 4096, 10
N_CORES = 8
RPC = (B * L) // N_CORES          # rows of h_english per core = 2048
OS = H // N_CORES                 # per-core weight contraction slice = 256
SB = B * S                        # flattened (batch, s) = 40
NP = RPC // 256                   # 256-row tile-pairs per core = 8
NH = H // 128                     # 128-wide h chunks = 16
F32 = mybir.dt.float32
BF16 = mybir.dt.bfloat16

KQ_SZ = 128 * NH * S              # 20480 floats per rs slice for kq
VO_SZ = S * H                     # 20480 floats per rs slice for vo
RS_SZ = KQ_SZ + VO_SZ             # 40960

AF = mybir.ActivationFunctionType
ALU = mybir.AluOpType


def build_graph():
    nc = bacc.Bacc(None, num_devices=N_CORES)

    hl_p = nc.declare_dram_parameter("hl_p", [128, NH * SB], BF16, isOutput=False)
    w_kT_p = nc.declare_dram_parameter("w_kT_p", [128, NH * OS], BF16, isOutput=False)
    w_q_p = nc.declare_dram_parameter("w_q_p", [128, 2 * H], BF16, isOutput=False)
    w_vT_p = nc.declare_dram_parameter("w_vT_p", [128, NH * OS], BF16, isOutput=False)
    w_oT_p = nc.declare_dram_parameter("w_oT_p", [128, 2 * H], BF16, isOutput=False)
    h_rows_p = nc.declare_dram_parameter(
        "h_rows_p", [128, NP * 2 * H], BF16, isOutput=False
    )
    h_T_p = nc.declare_dram_parameter(
        "h_T_p", [128, NP * 2 * NH * 128], BF16, isOutput=False
    )
    inv_alpha = nc.declare_dram_parameter("inv_alpha10", [S, 1], F32, isOutput=False)
    out = nc.declare_dram_parameter("out", [RPC, H], F32, isOutput=True)

    with tile.TileContext(nc) as tc, contextlib.ExitStack() as ctx:
        singles = ctx.enter_context(tc.tile_pool(name="singles", bufs=1))
        wpool = ctx.enter_context(tc.tile_pool(name="wpool", bufs=1))
        ppool = ctx.enter_context(tc.tile_pool(name="ppool", bufs=1))
        dpool = ctx.enter_context(tc.tile_pool(name="dram", bufs=1, space="DRAM"))
        opool = ctx.enter_context(tc.tile_pool(name="opool", bufs=6))
        sdpool = ctx.enter_context(tc.tile_pool(name="sdpool", bufs=3))
        spool = ctx.enter_context(tc.tile_pool(name="spool", bufs=2))
        pp_s = ctx.enter_context(tc.tile_pool(name="pp_s", bufs=2, space="PSUM"))
        pp_d = ctx.enter_context(tc.tile_pool(name="pp_d", bufs=2, space="PSUM"))
        pp_d2 = ctx.enter_context(tc.tile_pool(name="pp_d2", bufs=2, space="PSUM"))

        kq_T = singles.tile([128, NH, S], BF16)     # kq_T[p, c, s] = kq[s, 128c+p]
        vo_aug = singles.tile([S, H + 1], BF16)     # vo rows + 1/alpha column

        # ---------------- weight loads first (sync HWDGE, FIFO priority) ------
        def load_w(param, shape, tag):
            t = wpool.tile(shape, BF16, tag=tag)
            nc.sync.dma_start(
                out=t[:], in_=param[:].rearrange("p (c j) -> p c j", c=shape[1])
            )
            return t

        hl = load_w(hl_p, [128, NH, SB], "hl")
        w_kT = load_w(w_kT_p, [128, NH, OS], "wk")
        w_vT = load_w(w_vT_p, [128, NH, OS], "wv")
        w_q = load_w(w_q_p, [128, 2, H], "wq")
        w_oT = load_w(w_oT_p, [128, 2, H], "wo")

        # ---------------- h streams (sync HWDGE, behind weights) --------------
        h_rows = singles.tile([128, NP, 2, H], BF16)
        h_T = singles.tile([128, NP, 2 * NH, 128], BF16)
        for half in range(2):
            nc.sync.dma_start(
                out=h_rows[:, 4 * half : 4 * (half + 1), :, :],
                in_=h_rows_p[:, 8 * H * half : 8 * H * (half + 1)].rearrange(
                    "p (n g j) -> p n g j", n=4, g=2
                ),
            )
        for half in range(2):
            nc.sync.dma_start(
                out=h_T[:, 4 * half : 4 * (half + 1), :, :],
                in_=h_T_p[:, 4096 * 4 * half : 4096 * 4 * (half + 1)].rearrange(
                    "p (n c r) -> p n c r", n=4, c=2 * NH
                ),
            )

        # ---------------- prep: kq/vo partials on PE --------------------------
        k_T = ppool.tile([128, 2, SB], BF16, tag="kT")
        v_T = ppool.tile([128, 2, SB], BF16, tag="vT")
        for w_sb, dst in ((w_kT, k_T), (w_vT, v_T)):
            for oc in range(2):
                ps = pp_s.tile([128, SB], F32, tag="s")
                for hc in range(NH):
                    nc.tensor.matmul(
                        ps[:],
                        lhsT=w_sb[:, hc, 128 * oc : 128 * (oc + 1)],
                        rhs=hl[:, hc, :],
                        start=(hc == 0),
                        stop=(hc == NH - 1),
                    )
                nc.vector.tensor_copy(dst[:, oc, :], ps[:])

        kq_pT = ppool.tile([128, B, NH, S], BF16, tag="kqp")
        for hc in range(NH):
            ps = pp_s.tile([128, SB], F32, tag="s")
            for oc in range(2):
                nc.tensor.matmul(
                    ps[:],
                    lhsT=w_q[:, oc, 128 * hc : 128 * (hc + 1)],
                    rhs=k_T[:, oc, :],
                    start=(oc == 0),
                    stop=(oc == 1),
                )
            nc.vector.tensor_copy(
                kq_pT[:, :, hc, :], ps[:].rearrange("p (b s) -> p b s", b=B)
            )

        vo_p = ppool.tile([SB, H], BF16, tag="vop")
        for n4 in range(4):
            ps = pp_s.tile([SB, 512], F32, tag="s")
            for hc2 in range(2):
                nc.tensor.matmul(
                    ps[:],
                    lhsT=v_T[:, hc2, :],
                    rhs=w_oT[:, hc2, 512 * n4 : 512 * (n4 + 1)],
                    start=(hc2 == 0),
                    stop=(hc2 == 1),
                )
            nc.vector.tensor_copy(vo_p[:, 512 * n4 : 512 * (n4 + 1)], ps[:])

        rs_in = dpool.tile([N_CORES, RS_SZ], BF16, tag="rsi")
        rs_out = dpool.tile([1, RS_SZ], BF16, tag="rso")

        # ------------- rs writes (scalar), RS, readback (gpsimd) --------------
        rs_ap = rs_in[:]
        for b in range(B):
            kq_src = kq_pT[:, b, :, :]
            kq_bcast = bass_mod.AP(
                tensor=kq_src.tensor,
                offset=kq_src.offset,
                ap=[kq_src.ap[0], [0, 2], [1, NH * S]],
            )
            kq_dst = bass_mod.AP(
                tensor=rs_ap.tensor,
                offset=rs_ap.offset + 2 * b * RS_SZ,
                ap=[[NH * S, 128], [RS_SZ, 2], [1, NH * S]],
            )
            nc.scalar.dma_start(out=kq_dst, in_=kq_bcast)
            vo_src = vo_p[S * b : S * (b + 1), :]
            vo_bcast = bass_mod.AP(
                tensor=vo_src.tensor,
                offset=vo_src.offset,
                ap=[vo_src.ap[0], [0, 2], vo_src.ap[1]],
            )
            vo_dst = bass_mod.AP(
                tensor=rs_ap.tensor,
                offset=rs_ap.offset + 2 * b * RS_SZ + KQ_SZ,
                ap=[[H, S], [RS_SZ, 2], [1, H]],
            )
            nc.scalar.dma_start(out=vo_dst, in_=vo_bcast)

        nc.gpsimd.collective_compute(
            "ReduceScatter",
            mybir.AluOpType.add,
            replica_groups=[list(range(N_CORES))],
            ins=[rs_in[:].opt()],
            outs=[rs_out[:].opt()],
        )
        nc.gpsimd.dma_start(
            out=kq_T[:],
            in_=rs_out[0, :KQ_SZ].rearrange("(p c s) -> p c s", p=128, c=NH),
        )
        nc.gpsimd.dma_start(
            out=vo_aug[:, :H],
            in_=rs_out[0, KQ_SZ:].rearrange("(s o) -> s o", s=S),
        )
        nc.gpsimd.dma_start(out=vo_aug[:, H : H + 1], in_=inv_alpha[:])

        # ------------------------- compute loop (SW-pipelined) ----------------
        # 512-row groups (2 tile-pairs each); delta of group g emitted after
        # scores of group g+1 so the PE never waits on the Exp.
        NG = NP // 2  # 4 groups

        def emit_delta(g4, exp_sT):
            for t in range(4):
                p, g = 2 * g4 + t // 2, t % 2
                out_t = opool.tile([128, H], BF16, tag="out")
                exp_t = exp_sT[:, 128 * t : 128 * (t + 1)]
                ps_d2 = pp_d2.tile([128, 1], F32, tag="d2")
                nc.tensor.matmul(
                    ps_d2[:], lhsT=exp_t, rhs=vo_aug[:, H : H + 1],
                    start=True, stop=True,
                )
                r_scale = spool.tile([128, 1], F32, tag="rs")
                nc.vector.reciprocal(r_scale[:], ps_d2[:])

                for half in range(2):
                    ps_d = pp_d.tile([128, 1024], F32, tag="d")
                    for q in range(2):
                        n4 = 2 * half + q
                        nc.tensor.matmul(
                            ps_d[:, 512 * q : 512 * (q + 1)],
                            lhsT=exp_t,
                            rhs=vo_aug[:, 512 * n4 : 512 * (n4 + 1)],
                            start=True,
                            stop=True,
                        )
                    h_res = h_rows[:, p, g, 1024 * half : 1024 * (half + 1)]
                    if half == 0:
                        # out = delta * (alpha / sum exp) + h on DVE
                        nc.vector.scalar_tensor_tensor(
                            out_t[:, 1024 * half : 1024 * (half + 1)],
                            ps_d[:],
                            r_scale[:],
                            h_res,
                            op0=ALU.mult,
                            op1=ALU.add,
                        )
                    else:
                        # scaled copy on ScalarE (PSUM port), bf16 add on DVE
                        sd = sdpool.tile([128, 1024], BF16, tag="sd")
                        nc.scalar.activation(
                            sd[:], ps_d[:], AF.Copy, scale=r_scale[:]
                        )
                        nc.vector.tensor_tensor(
                            out_t[:, 1024 * half : 1024 * (half + 1)],
                            sd[:],
                            h_res,
                            op=ALU.add,
                        )
                # cast-store bf16 -> f32 rows (SWDGE)
                nc.gpsimd.dma_start(
                    out=out[512 * g4 + 128 * t : 512 * g4 + 128 * (t + 1), :],
                    in_=out_t[:],
                )

        prev = None
        for g4 in range(NG):
            hT_g = h_T[:, 2 * g4 : 2 * g4 + 2, :, :].rearrange(
                "p n (g c) r -> p c n g r", g=2
            )
            ps_s = pp_s.tile([S, 512], F32, tag="s")
            for hc in range(NH):
                nc.tensor.matmul(
                    ps_s[:],
                    lhsT=kq_T[:, hc, :],
                    rhs=hT_g[:, hc, :, :, :],
                    start=(hc == 0),
                    stop=(hc == NH - 1),
                )
            exp_sT = spool.tile([S, 512], BF16, tag="exp")
            nc.scalar.activation(
                exp_sT[:], ps_s[:], AF.Exp, scale=float(1.0 / np.sqrt(H))
            )
            if prev is not None:
                emit_delta(*prev)
            prev = (g4, exp_sT)
        emit_delta(*prev)

    nc.compile()
    return nc


_graph_cache = {}


def _get_graph():
    if "nc" not in _graph_cache:
        _graph_cache["nc"] = build_graph()
    return _graph_cache["nc"]


def _pack(x):
    """[C*128, J] f32 -> [128, C*J] partition-major packed bf16."""
    c = x.shape[0] // 128
    return np.ascontiguousarray(
        x.reshape(c, 128, x.shape[1]).transpose(1, 0, 2).reshape(128, -1)
    ).astype(ml_dtypes.bfloat16)


def _make_in_maps(inputs):
    h_english = np.ascontiguousarray(np.asarray(inputs["h_english"], dtype=np.float32))
    h_lojban = np.ascontiguousarray(np.asarray(inputs["h_lojban"], dtype=np.float32))
    w_q = np.asarray(inputs["w_q"], dtype=np.float32)
    w_k = np.asarray(inputs["w_k"], dtype=np.float32)
    w_v = np.asarray(inputs["w_v"], dtype=np.float32)
    w_o = np.asarray(inputs["w_o"], dtype=np.float32)
    alpha = float(np.asarray(inputs["alpha"], dtype=np.float32))

    h_flat = h_english.reshape(B * L, H)
    hl_p = _pack(np.ascontiguousarray(h_lojban.reshape(SB, H).T))
    inv_a = np.full((S, 1), 1.0 / alpha, dtype=np.float32)

    in_maps = []
    for i in range(N_CORES):
        sl = slice(OS * i, OS * (i + 1))
        hc = h_flat[RPC * i : RPC * (i + 1)]
        # rows: [p2, pair, g, :] = h row (256*pair + 128*g + p2)
        h_rows = np.ascontiguousarray(
            hc.reshape(NP, 2, 128, H).transpose(2, 0, 1, 3).reshape(128, -1)
        ).astype(ml_dtypes.bfloat16)
        # transposed: [p2, pair, g*16+hck, r] = h[256*pair+128*g+r, 128*hck+p2]
        h_T = np.ascontiguousarray(
            hc.reshape(NP, 2, 128, NH, 128).transpose(4, 0, 1, 3, 2).reshape(128, -1)
        ).astype(ml_dtypes.bfloat16)
        in_maps.append({
            "hl_p": hl_p,
            "w_kT_p": _pack(np.ascontiguousarray(w_k[sl, :].T)),
            "w_q_p": _pack(np.ascontiguousarray(w_q[sl, :])),
            "w_vT_p": _pack(np.ascontiguousarray(w_v[sl, :].T)),
            "w_oT_p": _pack(np.ascontiguousarray(w_o[:, sl].T)),
            "h_rows_p": h_rows,
            "h_T_p": h_T,
            "inv_alpha10": inv_a,
        })
    return in_maps


def kernel(**inputs):
    in_maps = _make_in_maps(inputs)
    nc = _get_graph()
    res = run_bass_kernel_spmd(nc, in_maps, core_ids=list(range(N_CORES)))
    out = np.concatenate([res.results[i]["out"] for i in range(N_CORES)], axis=0)
    return np.ascontiguousarray(out.reshape(B, L, H).astype(np.float32))
